# revision 2
# baseline (speedup 1.0000x reference)
"""AlignmentTable kernel for 8 Trainium2 NeuronCores.

Reference computation (N1 = N2 = 8192, VOCAB = 1024):
    eq[i,j]   = seq1[i] == seq2[j]
    ch0[i,j]  = eq ? pw_scores[seq1[i], seq2[j]] : 0        (padded to 8193x8193)
    out       = stack([ch0, gap, gap], axis=-1)             (8193, 8193, 3) f32

Where eq holds, pw_scores[seq1[i], seq2[j]] == pw_scores[v, v] — a diagonal
element — so the device only needs dval[i] = diag(pw_scores)[seq1[i]]:
    out[i,j,0] = (seq1[i] == seq2[j]) * dval[i]

Sharding: rows split across 8 cores (1024 rows each); seq2 replicated. Each
core materializes its 1024x8193x3 slab (~100 MB) — a pure HBM-write problem
(~805 MB total) bounded by the 16 SDMA engines (~27 GiB/s each, ~431 GB/s
per core when the HBM stack is uncontended).

Per-core layout is PLANAR (ch0 plane + two constant gap planes; the host
interleaves channels during unshard).  That splits the store traffic into
two independent streams on the two HWDGE queues:

  * qAct (nc.scalar): the computed ch0 plane — 16 chunk DMAs, each gated on
    a VectorE tensor_scalar (seq2==tok)*dval into a rotating buffer.
  * qSP (nc.sync): the two constant planes — DMAs that only depend on a
    one-time gap fill of a single (128, 8193) buffer, so this queue is
    never blocked and keeps all 16 SDMA engines saturated while the ch0
    pipeline ramps or hiccups.

DMA engine skew: descriptors of one DMA are assigned to SDMA engines
starting at engine 0 (measured), and engine 15 is a persistent ~26%-slow
straggler on some cores.  Constant-plane DMAs therefore use 63-row groups
(engines 0-14 carry 4 rows, engine 15 carries 3), cutting engine 15's
per-core load to ~84% of fair share: near-optimal when it is slow, ~1%
overhead when it is healthy.

The trailing output row 8192 (constant) is written on the host.
"""

import numpy as np

N1 = 8192
N2 = 8192
NCORES = 8
P = 128
ROWS_PER_CORE = N1 // NCORES          # 1024
RTILES = ROWS_PER_CORE // P           # 8
NJ = N2 + 1                           # 8193 output columns
MMW = 512                             # matmul free-dim width (one PSUM bank)
NMM = (NJ + MMW - 1) // MMW           # 17 broadcast matmuls
CHUNKS = ((0, 4097), (4097, 4096))    # ch0 column chunks per row tile
NBUF = 3
# CB fill column ranges (DVE, in order); const starters are split the same
# way so the first stores launch as soon as the first fill lands.
FILLS = ((0, 2048), (2048, 4608), (4608, NJ))
# Constant-plane row groups: 16x63 + 1x16 = 1024 rows per plane.
CROWS = [(g * 63, 63) for g in range(16)] + [(1008, 16)]
_cache = {}


def _build_nc():
    import concourse.bacc as bacc
    import concourse.mybir as mybir
    from concourse.tile import TileContext

    f32 = mybir.dt.float32
    f16 = mybir.dt.float16
    nc = bacc.Bacc(None, target_bir_lowering=False)

    # meta columns: [0:8] tok per row-tile, [8:16] dval per row-tile, [16] gap
    meta = nc.dram_tensor("meta", [P, 2 * RTILES + 1], f32, kind="ExternalInput")
    # seq2 tokens in fp16 (0..1023 and the -1 pad are exact).
    s2 = nc.dram_tensor("s2", [NJ], f16, kind="ExternalInput")
    out0 = nc.dram_tensor("out0", [ROWS_PER_CORE, NJ], f32, kind="ExternalOutput")
    outc = nc.dram_tensor("outc", [2 * ROWS_PER_CORE, NJ], f32, kind="ExternalOutput")

    with TileContext(nc) as tc:
        with (
            tc.tile_pool(name="sbuf", bufs=1) as pool,
            tc.tile_pool(name="psum", bufs=2, space="PSUM") as psum,
        ):
            META = pool.tile([P, 2 * RTILES + 1], f32, tag="meta")
            ONES = pool.tile([1, P], f16, tag="ones")
            S2ROW = pool.tile([1, NJ], f16, tag="s2row")
            S2B = pool.tile([P, NJ], f32, tag="s2b")
            CB = pool.tile([P, NJ], f32, tag="cb")
            BUFS = [
                pool.tile([P, CHUNKS[0][1]], f32, tag=f"buf{i}", name=f"buf{i}")
                for i in range(NBUF)
            ]
            GAP = META[:, 2 * RTILES : 2 * RTILES + 1]

            # Input loads: meta via ACT HWDGE, seq2 row via SP HWDGE.
            nc.scalar.dma_start(out=META[:], in_=meta[:])
            nc.sync.dma_start(out=S2ROW[:], in_=s2[None, :])
            nc.gpsimd.memset(ONES[:], 1.0)

            # Gap fill of the constant source buffer (VectorE, 3 chunks so
            # the first constant stores can start after ~1.5 us).
            for lo, hi in FILLS:
                nc.vector.tensor_scalar(
                    out=CB[:, lo:hi],
                    in0=GAP.to_broadcast((P, hi - lo)),
                    scalar1=1.0,
                    scalar2=None,
                    op0=mybir.AluOpType.mult,
                )

            # Broadcast seq2 across partitions: S2B[p, j] = s2[j] via
            # ones(128) outer-product matmuls; PSUM -> SBUF copies on DVE
            # (keeps ACT free to push ch0 store descriptors promptly).
            for k in range(NMM):
                lo = k * MMW
                w = min(MMW, NJ - lo)
                ps = psum.tile([P, MMW], f32, tag="ps", name="ps")
                nc.tensor.matmul(
                    ps[:, :w], ONES[:], S2ROW[:, lo : lo + w], start=True, stop=True
                )
                nc.vector.tensor_scalar(
                    out=S2B[:, lo : lo + w],
                    in0=ps[:, :w],
                    scalar1=1.0,
                    scalar2=None,
                    op0=mybir.AluOpType.mult,
                )

            # ch0 plane: per (row-tile, column-chunk) one VectorE
            #   (seq2_chunk == tok_row) * dval_row
            # into a rotating buffer, then a ~2.1 MB store on qAct.
            bi = 0
            for rt in range(RTILES):
                for cs, w in CHUNKS:
                    b = BUFS[bi % NBUF]
                    bi += 1
                    nc.vector.tensor_scalar(
                        out=b[:, :w],
                        in0=S2B[:, cs : cs + w],
                        scalar1=META[:, rt : rt + 1],
                        scalar2=META[:, RTILES + rt : RTILES + rt + 1],
                        op0=mybir.AluOpType.is_equal,
                        op1=mybir.AluOpType.mult,
                    )
                    nc.scalar.dma_start(
                        out=out0[rt * P : (rt + 1) * P, cs : cs + w],
                        in_=b[:, :w],
                    )

            # Constant planes on qSP.  First two 63-row groups of plane 1
            # are column-split along the fill chunks (early starters); the
            # rest are full-width.
            def cstore(r0, nr, lo, hi):
                nc.sync.dma_start(
                    out=outc[r0 : r0 + nr, lo:hi], in_=CB[:nr, lo:hi]
                )

            for g in range(2):
                for lo, hi in FILLS:
                    cstore(g * 63, 63, lo, hi)
            for r0, nr in CROWS[2:]:
                cstore(r0, nr, 0, NJ)
            for r0, nr in CROWS:
                cstore(ROWS_PER_CORE + r0, nr, 0, NJ)
    nc.compile()
    return nc


def _get_nc():
    if "nc" not in _cache:
        _cache["nc"] = _build_nc()
    return _cache["nc"]


def _prep_in_maps(encoded_seq1, encoded_seq2, pw_scores, gap_score):
    seq1 = np.asarray(encoded_seq1).astype(np.int64)
    seq2 = np.asarray(encoded_seq2).astype(np.int64)
    pw = np.asarray(pw_scores).astype(np.float32)
    gapf = np.float32(np.asarray(gap_score))

    dvals = pw.diagonal().astype(np.float32)[seq1]      # (8192,)
    seq1f = seq1.astype(np.float32)
    s2pad = np.empty(NJ, np.float16)
    s2pad[:N2] = seq2.astype(np.float16)                # 0..1023: exact in fp16
    s2pad[N2] = -1.0                                    # never matches a token

    in_maps = []
    for r in range(NCORES):
        lo, hi = r * ROWS_PER_CORE, (r + 1) * ROWS_PER_CORE
        meta = np.empty((P, 2 * RTILES + 1), np.float32)
        meta[:, :RTILES] = seq1f[lo:hi].reshape(RTILES, P).T
        meta[:, RTILES : 2 * RTILES] = dvals[lo:hi].reshape(RTILES, P).T
        meta[:, 2 * RTILES] = gapf
        in_maps.append({"s2": s2pad, "meta": meta})
    return in_maps, gapf


def _assemble(results, gapf):
    out = np.empty((N1 + 1, NJ, 3), np.float32)
    for r in range(NCORES):
        sl = slice(r * ROWS_PER_CORE, (r + 1) * ROWS_PER_CORE)
        res = results[r]
        out[sl, :, 0] = res["out0"]
        cc = res["outc"].reshape(2, ROWS_PER_CORE, NJ)
        out[sl, :, 1] = cc[0]
        out[sl, :, 2] = cc[1]
    out[N1, :, 0] = 0.0
    out[N1, :, 1] = gapf
    out[N1, :, 2] = gapf
    return out


def run(encoded_seq1, encoded_seq2, pw_scores, gap_score, **spmd_kwargs):
    """Full pipeline; extra kwargs (trace=..., tmpdir=...) are forwarded to
    run_bass_kernel_spmd. Returns (output, BassKernelResults)."""
    from concourse.bass_utils import run_bass_kernel_spmd

    in_maps, gapf = _prep_in_maps(encoded_seq1, encoded_seq2, pw_scores, gap_score)
    res = run_bass_kernel_spmd(
        _get_nc(), in_maps, core_ids=list(range(NCORES)), **spmd_kwargs
    )
    return _assemble(res.results, gapf), res


def kernel(encoded_seq1, encoded_seq2, pw_scores, gap_score):
    out, _ = run(encoded_seq1, encoded_seq2, pw_scores, gap_score)
    return out


# revision 5
# speedup vs baseline: 2.0425x; 2.0425x over previous
"""AlignmentTable kernel for 8 Trainium2 NeuronCores.

Reference computation (N1 = N2 = 8192, VOCAB = 1024):
    eq[i,j]   = seq1[i] == seq2[j]
    ch0[i,j]  = eq ? pw_scores[seq1[i], seq2[j]] : 0        (padded to 8193x8193)
    out       = stack([ch0, gap, gap], axis=-1)             (8193, 8193, 3) f32

Where eq holds, pw_scores[seq1[i], seq2[j]] == pw_scores[v, v] — a diagonal
element — so the device only needs dval[i] = diag(pw_scores)[seq1[i]]:
    out[i,j,0] = (seq1[i] == seq2[j]) * dval[i]

Sharding: rows split across 8 cores (1024 rows each); seq2 replicated. Each
core materializes its 1024x8193x3 slab (~100 MB) — a pure HBM-write problem
(~805 MB total) bounded by the 16 SDMA engines (~27 GiB/s each, ~431 GB/s
per core when the HBM stack is uncontended).

Per-core layout is PLANAR (ch0 plane + two constant gap planes; the host
interleaves channels during unshard).  That splits the store traffic into
two independent streams on the two HWDGE queues:

  * qAct (nc.scalar): the computed ch0 plane — 16 chunk DMAs, each gated on
    a VectorE tensor_scalar (seq2==tok)*dval into a rotating buffer.
  * qSP (nc.sync): the two constant planes — DMAs that only depend on a
    one-time gap fill of a single (128, 8193) buffer, so this queue is
    never blocked and keeps all 16 SDMA engines saturated while the ch0
    pipeline ramps or hiccups.

Every store is a (128 partitions, column-chunk) DMA: measured, this shape
sprays descriptors uniformly over all 16 SDMA engines and reads all 16
SBUF AXI ports.  Sub-128-partition sources concentrate on few ports
(~27 GB/s each) and full-width stores whose DRAM dest collapses to one
contiguous run get a pathological 9-engine descriptor assignment — both
measured 2x slowdowns.

The trailing output row 8192 (constant) is written on the host.
"""

import numpy as np

N1 = 8192
N2 = 8192
NCORES = 8
P = 128
ROWS_PER_CORE = N1 // NCORES          # 1024
RTILES = ROWS_PER_CORE // P           # 8
NJ = N2 + 1                           # 8193 output columns
MMW = 512                             # matmul free-dim width (one PSUM bank)
NMM = (NJ + MMW - 1) // MMW           # 17 broadcast matmuls
CHUNKS = ((0, 4097), (4097, 4096))    # ch0 column chunks per row tile
NBUF = 3
# CB fill column ranges (DVE, in order); the first const row-tile is split
# the same way so the first stores launch as soon as the first fill lands.
FILLS = ((0, 2048), (2048, 4608), (4608, NJ))
_cache = {}


def _build_nc():
    import concourse.bacc as bacc
    import concourse.mybir as mybir
    from concourse.tile import TileContext

    f32 = mybir.dt.float32
    f16 = mybir.dt.float16
    nc = bacc.Bacc(None, target_bir_lowering=False)

    # meta columns: [0:8] tok per row-tile, [8:16] dval per row-tile, [16] gap
    meta = nc.dram_tensor("meta", [P, 2 * RTILES + 1], f32, kind="ExternalInput")
    # seq2 tokens in fp16 (0..1023 and the -1 pad are exact).
    s2 = nc.dram_tensor("s2", [NJ], f16, kind="ExternalInput")
    out0 = nc.dram_tensor("out0", [ROWS_PER_CORE, NJ], f32, kind="ExternalOutput")
    outc = nc.dram_tensor("outc", [2 * ROWS_PER_CORE, NJ], f32, kind="ExternalOutput")

    with TileContext(nc) as tc:
        with (
            tc.tile_pool(name="sbuf", bufs=1) as pool,
            tc.tile_pool(name="psum", bufs=2, space="PSUM") as psum,
        ):
            META = pool.tile([P, 2 * RTILES + 1], f32, tag="meta")
            ONES = pool.tile([1, P], f16, tag="ones")
            S2ROW = pool.tile([1, NJ], f16, tag="s2row")
            S2B = pool.tile([P, NJ], f32, tag="s2b")
            CB = pool.tile([P, NJ], f32, tag="cb")
            BUFS = [
                pool.tile([P, CHUNKS[0][1]], f32, tag=f"buf{i}", name=f"buf{i}")
                for i in range(NBUF)
            ]
            GAP = META[:, 2 * RTILES : 2 * RTILES + 1]

            # Input loads: meta via ACT HWDGE, seq2 row via SP HWDGE.
            nc.scalar.dma_start(out=META[:], in_=meta[:])
            nc.sync.dma_start(out=S2ROW[:], in_=s2[None, :])
            nc.gpsimd.memset(ONES[:], 1.0)

            # Gap fill of the constant source buffer (VectorE, 3 chunks so
            # the first constant stores can start after ~1.5 us).
            for lo, hi in FILLS:
                nc.vector.tensor_scalar(
                    out=CB[:, lo:hi],
                    in0=GAP.to_broadcast((P, hi - lo)),
                    scalar1=1.0,
                    scalar2=None,
                    op0=mybir.AluOpType.mult,
                )

            # Broadcast seq2 across partitions: S2B[p, j] = s2[j] via
            # ones(128) outer-product matmuls; PSUM -> SBUF copies on DVE
            # (keeps ACT free to push ch0 store descriptors promptly).
            for k in range(NMM):
                lo = k * MMW
                w = min(MMW, NJ - lo)
                ps = psum.tile([P, MMW], f32, tag="ps", name="ps")
                nc.tensor.matmul(
                    ps[:, :w], ONES[:], S2ROW[:, lo : lo + w], start=True, stop=True
                )
                nc.vector.tensor_scalar(
                    out=S2B[:, lo : lo + w],
                    in0=ps[:, :w],
                    scalar1=1.0,
                    scalar2=None,
                    op0=mybir.AluOpType.mult,
                )

            # ch0 plane: per (row-tile, column-chunk) one VectorE
            #   (seq2_chunk == tok_row) * dval_row
            # into a rotating buffer, then a ~2.1 MB store on qAct.
            bi = 0
            for rt in range(RTILES):
                for cs, w in CHUNKS:
                    b = BUFS[bi % NBUF]
                    bi += 1
                    nc.vector.tensor_scalar(
                        out=b[:, :w],
                        in0=S2B[:, cs : cs + w],
                        scalar1=META[:, rt : rt + 1],
                        scalar2=META[:, RTILES + rt : RTILES + rt + 1],
                        op0=mybir.AluOpType.is_equal,
                        op1=mybir.AluOpType.mult,
                    )
                    nc.scalar.dma_start(
                        out=out0[rt * P : (rt + 1) * P, cs : cs + w],
                        in_=b[:, :w],
                    )

            # Constant planes on qSP: 16 row-tiles x 2 column chunks.  The
            # first two tiles are split along the fill chunks instead so
            # the first stores launch ~1.5 us after the gap value lands.
            def cstore(r0, lo, hi):
                nc.sync.dma_start(
                    out=outc[r0 : r0 + P, lo:hi], in_=CB[:, lo:hi]
                )

            for t in range(2):
                for lo, hi in FILLS:
                    cstore(t * P, lo, hi)
            for t in range(2, 2 * RTILES):
                for cs, w in CHUNKS:
                    cstore(t * P, cs, cs + w)
    nc.compile()
    return nc


def _get_nc():
    if "nc" not in _cache:
        _cache["nc"] = _build_nc()
    return _cache["nc"]


def _prep_in_maps(encoded_seq1, encoded_seq2, pw_scores, gap_score):
    seq1 = np.asarray(encoded_seq1).astype(np.int64)
    seq2 = np.asarray(encoded_seq2).astype(np.int64)
    pw = np.asarray(pw_scores).astype(np.float32)
    gapf = np.float32(np.asarray(gap_score))

    dvals = pw.diagonal().astype(np.float32)[seq1]      # (8192,)
    seq1f = seq1.astype(np.float32)
    s2pad = np.empty(NJ, np.float16)
    s2pad[:N2] = seq2.astype(np.float16)                # 0..1023: exact in fp16
    s2pad[N2] = -1.0                                    # never matches a token

    in_maps = []
    for r in range(NCORES):
        lo, hi = r * ROWS_PER_CORE, (r + 1) * ROWS_PER_CORE
        meta = np.empty((P, 2 * RTILES + 1), np.float32)
        meta[:, :RTILES] = seq1f[lo:hi].reshape(RTILES, P).T
        meta[:, RTILES : 2 * RTILES] = dvals[lo:hi].reshape(RTILES, P).T
        meta[:, 2 * RTILES] = gapf
        in_maps.append({"s2": s2pad, "meta": meta})
    return in_maps, gapf


def _assemble(results, gapf):
    out = np.empty((N1 + 1, NJ, 3), np.float32)
    for r in range(NCORES):
        sl = slice(r * ROWS_PER_CORE, (r + 1) * ROWS_PER_CORE)
        res = results[r]
        out[sl, :, 0] = res["out0"]
        cc = res["outc"].reshape(2, ROWS_PER_CORE, NJ)
        out[sl, :, 1] = cc[0]
        out[sl, :, 2] = cc[1]
    out[N1, :, 0] = 0.0
    out[N1, :, 1] = gapf
    out[N1, :, 2] = gapf
    return out


def run(encoded_seq1, encoded_seq2, pw_scores, gap_score, **spmd_kwargs):
    """Full pipeline; extra kwargs (trace=..., tmpdir=...) are forwarded to
    run_bass_kernel_spmd. Returns (output, BassKernelResults)."""
    from concourse.bass_utils import run_bass_kernel_spmd

    in_maps, gapf = _prep_in_maps(encoded_seq1, encoded_seq2, pw_scores, gap_score)
    res = run_bass_kernel_spmd(
        _get_nc(), in_maps, core_ids=list(range(NCORES)), **spmd_kwargs
    )
    return _assemble(res.results, gapf), res


def kernel(encoded_seq1, encoded_seq2, pw_scores, gap_score):
    out, _ = run(encoded_seq1, encoded_seq2, pw_scores, gap_score)
    return out


# revision 7
# speedup vs baseline: 2.0706x; 1.0138x over previous
"""AlignmentTable kernel for 8 Trainium2 NeuronCores.

Reference computation (N1 = N2 = 8192, VOCAB = 1024):
    eq[i,j]   = seq1[i] == seq2[j]
    ch0[i,j]  = eq ? pw_scores[seq1[i], seq2[j]] : 0        (padded to 8193x8193)
    out       = stack([ch0, gap, gap], axis=-1)             (8193, 8193, 3) f32

Where eq holds, pw_scores[seq1[i], seq2[j]] == pw_scores[v, v] — a diagonal
element — so the device only needs dval[i] = diag(pw_scores)[seq1[i]]:
    out[i,j,0] = (seq1[i] == seq2[j]) * dval[i]

Sharding: rows split across 8 cores (1024 rows each); seq2 replicated. Each
core materializes its 1024x8193x3 slab (~100 MB) — a pure HBM-write problem
(~805 MB total) bounded by the 16 SDMA engines (~27 GiB/s each, ~431 GB/s
per core when the HBM stack is uncontended).

Per-core layout is PLANAR (ch0 plane + two constant gap planes; the host
interleaves channels during unshard).  That splits the store traffic into
two independent streams on the two HWDGE queues:

  * qAct (nc.scalar): the computed ch0 plane — 16 chunk DMAs, each gated on
    a VectorE tensor_scalar (seq2==tok)*dval into a rotating buffer.
  * qSP (nc.sync): the two constant planes — DMAs that only depend on a
    one-time gap fill of a single (128, 8193) buffer, so this queue is
    never blocked and keeps all 16 SDMA engines saturated while the ch0
    pipeline ramps or hiccups.

Every store is a (128 partitions, column-chunk) DMA: measured, this shape
sprays descriptors uniformly over all 16 SDMA engines and reads all 16
SBUF AXI ports.  Sub-128-partition sources concentrate on few ports
(~27 GB/s each) and full-width stores whose DRAM dest collapses to one
contiguous run get a pathological 9-engine descriptor assignment — both
measured 2x slowdowns.

The trailing output row 8192 (constant) is written on the host.
"""

import numpy as np

N1 = 8192
N2 = 8192
NCORES = 8
P = 128
ROWS_PER_CORE = N1 // NCORES          # 1024
RTILES = ROWS_PER_CORE // P           # 8
NJ = N2 + 1                           # 8193 output columns
MMW = 512                             # matmul free-dim width (one PSUM bank)
NMM = (NJ + MMW - 1) // MMW           # 17 broadcast matmuls
CHUNKS = ((0, 4097), (4097, 4096))    # ch0 column chunks per row tile
NBUF = 3
# CB fill column ranges (DVE, in order); the first const row-tiles are
# split the same way so the first stores launch as soon as the first
# (tiny) fill lands.
FILLS = ((0, 512), (512, 2048), (2048, 4608), (4608, NJ))
_cache = {}


def _build_nc():
    import concourse.bacc as bacc
    import concourse.mybir as mybir
    from concourse.tile import TileContext

    f32 = mybir.dt.float32
    f16 = mybir.dt.float16
    nc = bacc.Bacc(None, target_bir_lowering=False)

    # meta columns: [0:8] tok per row-tile, [8:16] dval per row-tile, [16] gap
    meta = nc.dram_tensor("meta", [P, 2 * RTILES + 1], f32, kind="ExternalInput")
    # seq2 tokens in fp16 (0..1023 and the -1 pad are exact).
    s2 = nc.dram_tensor("s2", [NJ], f16, kind="ExternalInput")
    out0 = nc.dram_tensor("out0", [ROWS_PER_CORE, NJ], f32, kind="ExternalOutput")
    outc = nc.dram_tensor("outc", [2 * ROWS_PER_CORE, NJ], f32, kind="ExternalOutput")

    with TileContext(nc) as tc:
        with (
            tc.tile_pool(name="sbuf", bufs=1) as pool,
            tc.tile_pool(name="psum", bufs=2, space="PSUM") as psum,
        ):
            META = pool.tile([P, 2 * RTILES + 1], f32, tag="meta")
            ONES = pool.tile([1, P], f16, tag="ones")
            S2ROW = pool.tile([1, NJ], f16, tag="s2row")
            S2B = pool.tile([P, NJ], f32, tag="s2b")
            CB = pool.tile([P, NJ], f32, tag="cb")
            BUFS = [
                pool.tile([P, CHUNKS[0][1]], f32, tag=f"buf{i}", name=f"buf{i}")
                for i in range(NBUF)
            ]
            GAP = META[:, 2 * RTILES : 2 * RTILES + 1]

            # Input loads: meta via ACT HWDGE, seq2 row via SP HWDGE.
            nc.scalar.dma_start(out=META[:], in_=meta[:])
            nc.sync.dma_start(out=S2ROW[:], in_=s2[None, :])
            nc.gpsimd.memset(ONES[:], 1.0)

            # Gap fill of the constant source buffer (VectorE, 3 chunks so
            # the first constant stores can start after ~1.5 us).
            for lo, hi in FILLS:
                nc.vector.tensor_scalar(
                    out=CB[:, lo:hi],
                    in0=GAP.to_broadcast((P, hi - lo)),
                    scalar1=1.0,
                    scalar2=None,
                    op0=mybir.AluOpType.mult,
                )

            # Broadcast seq2 across partitions: S2B[p, j] = s2[j] via
            # ones(128) outer-product matmuls; PSUM -> SBUF copies on DVE
            # (keeps ACT free to push ch0 store descriptors promptly).
            for k in range(NMM):
                lo = k * MMW
                w = min(MMW, NJ - lo)
                ps = psum.tile([P, MMW], f32, tag="ps", name="ps")
                nc.tensor.matmul(
                    ps[:, :w], ONES[:], S2ROW[:, lo : lo + w], start=True, stop=True
                )
                nc.vector.tensor_scalar(
                    out=S2B[:, lo : lo + w],
                    in0=ps[:, :w],
                    scalar1=1.0,
                    scalar2=None,
                    op0=mybir.AluOpType.mult,
                )

            # ch0 plane: per (row-tile, column-chunk) one VectorE
            #   (seq2_chunk == tok_row) * dval_row
            # into a rotating buffer, then a ~2.1 MB store on qAct.
            bi = 0
            for rt in range(RTILES):
                for cs, w in CHUNKS:
                    b = BUFS[bi % NBUF]
                    bi += 1
                    nc.vector.tensor_scalar(
                        out=b[:, :w],
                        in0=S2B[:, cs : cs + w],
                        scalar1=META[:, rt : rt + 1],
                        scalar2=META[:, RTILES + rt : RTILES + rt + 1],
                        op0=mybir.AluOpType.is_equal,
                        op1=mybir.AluOpType.mult,
                    )
                    nc.scalar.dma_start(
                        out=out0[rt * P : (rt + 1) * P, cs : cs + w],
                        in_=b[:, :w],
                    )

            # Constant planes on qSP.  The first four row-tiles are split
            # along the fill chunks (~4 MB of store work unlocked ~1.5 us
            # after the gap value lands, bridging the ch0 pipeline ramp);
            # the rest are single full-width (128, 8193) stores.
            def cstore(r0, lo, hi):
                nc.sync.dma_start(
                    out=outc[r0 : r0 + P, lo:hi], in_=CB[:, lo:hi]
                )

            NSPLIT = 4
            for lo, hi in FILLS:
                for t in range(NSPLIT):
                    cstore(t * P, lo, hi)
            for t in range(NSPLIT, 2 * RTILES):
                cstore(t * P, 0, NJ)
    nc.compile()
    return nc


def _get_nc():
    if "nc" not in _cache:
        _cache["nc"] = _build_nc()
    return _cache["nc"]


def _prep_in_maps(encoded_seq1, encoded_seq2, pw_scores, gap_score):
    seq1 = np.asarray(encoded_seq1).astype(np.int64)
    seq2 = np.asarray(encoded_seq2).astype(np.int64)
    pw = np.asarray(pw_scores).astype(np.float32)
    gapf = np.float32(np.asarray(gap_score))

    dvals = pw.diagonal().astype(np.float32)[seq1]      # (8192,)
    seq1f = seq1.astype(np.float32)
    s2pad = np.empty(NJ, np.float16)
    s2pad[:N2] = seq2.astype(np.float16)                # 0..1023: exact in fp16
    s2pad[N2] = -1.0                                    # never matches a token

    in_maps = []
    for r in range(NCORES):
        lo, hi = r * ROWS_PER_CORE, (r + 1) * ROWS_PER_CORE
        meta = np.empty((P, 2 * RTILES + 1), np.float32)
        meta[:, :RTILES] = seq1f[lo:hi].reshape(RTILES, P).T
        meta[:, RTILES : 2 * RTILES] = dvals[lo:hi].reshape(RTILES, P).T
        meta[:, 2 * RTILES] = gapf
        in_maps.append({"s2": s2pad, "meta": meta})
    return in_maps, gapf


def _assemble(results, gapf):
    out = np.empty((N1 + 1, NJ, 3), np.float32)
    for r in range(NCORES):
        sl = slice(r * ROWS_PER_CORE, (r + 1) * ROWS_PER_CORE)
        res = results[r]
        out[sl, :, 0] = res["out0"]
        cc = res["outc"].reshape(2, ROWS_PER_CORE, NJ)
        out[sl, :, 1] = cc[0]
        out[sl, :, 2] = cc[1]
    out[N1, :, 0] = 0.0
    out[N1, :, 1] = gapf
    out[N1, :, 2] = gapf
    return out


def run(encoded_seq1, encoded_seq2, pw_scores, gap_score, **spmd_kwargs):
    """Full pipeline; extra kwargs (trace=..., tmpdir=...) are forwarded to
    run_bass_kernel_spmd. Returns (output, BassKernelResults)."""
    from concourse.bass_utils import run_bass_kernel_spmd

    in_maps, gapf = _prep_in_maps(encoded_seq1, encoded_seq2, pw_scores, gap_score)
    res = run_bass_kernel_spmd(
        _get_nc(), in_maps, core_ids=list(range(NCORES)), **spmd_kwargs
    )
    return _assemble(res.results, gapf), res


def kernel(encoded_seq1, encoded_seq2, pw_scores, gap_score):
    out, _ = run(encoded_seq1, encoded_seq2, pw_scores, gap_score)
    return out
